# revision 15
# baseline (speedup 1.0000x reference)
"""Trainium2 Bass kernel for CascadedLoRALinear4bit.

Computes out[b,s,o] = x @ W_base^T + b_base + scaling * (x @ A^T) @ B^T
with scaling == rank/alpha == 1.0.

Strategy:
  - Algebraic fold (exact): out = x @ (W_base + B @ A)^T + b_base.
    The fold is computed on host in fp32 (0.5 GFLOP, negligible).
  - Data-parallel over tokens: the 4*4096 = 16384 tokens are sharded
    8 ways (2048 tokens per NeuronCore). W_eff^T and bias are
    replicated to all cores. No collectives needed.
  - Per core: out_c^T[4096, 2048] = W_eff @ x_c^T + bias, tiled for
    the PE in bf16 with fp32 PSUM accumulation:
      * x_c^T stays fully resident in SBUF (16 MiB bf16), loaded once.
      * W_eff^T streams through as the stationary operand; each
        stationary tile is reused for 4 moving x chunks.
      * Output is computed transposed (o on partitions) so the bias is
        a per-partition scalar added by the DVE on PSUM eviction.
  - PE roofline: 4096 matmuls x [128x128]@[128x512] bf16.

Layouts (d = contraction dim on partitions everywhere):
  xT  [128, 4, 32, 512]  xT[p,mi,k,s] = x_c[mi*512+s, k*128+p]     (bf16)
  wT  [128, 32, 32, 128] wT[p,nO,k,o] = W_eff[nO*128+o, k*128+p]   (bf16)
  bias[128, 32]          bias[p,nO]   = b_base[nO*128+p]           (f32)
  out [128, 32, 4, 512]  out[p,nO,mi,s] = out_c[mi*512+s, nO*128+p] (f32)
"""

import sys

if "/opt/trn_rl_repo" not in sys.path:
    sys.path.insert(0, "/opt/trn_rl_repo")

import numpy as np
import ml_dtypes

import concourse.bass as bass
import concourse.mybir as mybir
import concourse.tile as tile
from concourse import bacc
from concourse.bass_utils import run_bass_kernel_spmd

# Problem dims (hardcoded per contract)
BATCH, SEQ, D_IN, D_OUT = 4, 4096, 4096, 4096
SCALING = 1.0  # rank / alpha = 16 / 16

N_CORES = 8
P = 128
S_PER_CORE = BATCH * SEQ // N_CORES  # 2048
KO = D_IN // P                       # 32 contraction tiles
S_TILE = 512
MI = S_PER_CORE // S_TILE            # 4 moving (token) chunks
NO = D_OUT // P                      # 32 output-row blocks

BF16 = mybir.dt.bfloat16
F32 = mybir.dt.float32

_compiled = {}


def _build_program(mi_n=MI, no_n=NO, ko=KO, s_tile=S_TILE):
    nc = bacc.Bacc(None, target_bir_lowering=False)

    xT = nc.declare_dram_parameter("xT", [P, mi_n, ko, s_tile], BF16, isOutput=False)
    wT = nc.declare_dram_parameter("wT", [P, no_n, ko, P], BF16, isOutput=False)
    bias_d = nc.declare_dram_parameter("bias", [P, no_n], F32, isOutput=False)
    out_d = nc.declare_dram_parameter("out", [P, no_n, mi_n, s_tile], F32, isOutput=True)

    with tile.TileContext(nc) as tc:
        with (
            tc.tile_pool(name="xres", bufs=1) as x_pool,
            tc.tile_pool(name="wt", bufs=3) as wt_pool,
            tc.tile_pool(name="bias", bufs=1) as bias_pool,
            tc.tile_pool(name="o", bufs=8) as out_pool,
            tc.tile_pool(name="psum", bufs=2, space="PSUM") as psum_pool,
        ):
            bias_t = bias_pool.tile([P, no_n], F32)
            nc.sync.dma_start(out=bias_t[:], in_=bias_d[:])

            # First stationary block, then x_c^T preload in k-major chunk
            # order so chunks land in the order the nO=0 k-loop consumes
            # them (x stays fully resident for all later nO iterations).
            K_CHUNK = min(2, ko)
            # Split wt0 so the first matmul only waits on its first two
            # k-slices (64 KiB) instead of the whole 1 MiB block.
            wt0 = wt_pool.tile([P, ko, P], BF16, name="wt")
            w_split = min(2, ko)
            nc.sync.dma_start(out=wt0[:, :w_split, :], in_=wT[:, 0, :w_split, :])

            xres = [x_pool.tile([P, ko, s_tile], BF16, name=f"x{mi}")
                    for mi in range(mi_n)]
            for kc in range(0, ko, K_CHUNK):
                for mi in range(mi_n):
                    nc.sync.dma_start(
                        out=xres[mi][:, kc:kc + K_CHUNK, :],
                        in_=xT[:, mi, kc:kc + K_CHUNK, :],
                    )
                if kc == 0 and w_split < ko:
                    nc.sync.dma_start(
                        out=wt0[:, w_split:, :], in_=wT[:, 0, w_split:, :]
                    )

            for n in range(no_n):
                if n == 0:
                    wt_blk = wt0
                else:
                    wt_blk = wt_pool.tile([P, ko, P], BF16, name="wt")
                    nc.sync.dma_start(out=wt_blk[:], in_=wT[:, n, :, :])
                pss = [psum_pool.tile([P, s_tile], F32, name=f"ps{mi}")
                       for mi in range(mi_n)]
                for k in range(ko):
                    for mi in range(mi_n):
                        nc.tensor.matmul(
                            pss[mi][:],
                            lhsT=wt_blk[:, k, :],
                            rhs=xres[mi][:, k, :],
                            start=(k == 0),
                            stop=(k == ko - 1),
                        )
                for mi in range(mi_n):
                    ot = out_pool.tile([P, s_tile], F32)
                    nc.vector.tensor_scalar_add(ot[:], pss[mi][:], bias_t[:, n:n + 1])
                    nc.sync.dma_start(out=out_d[:, n, mi, :], in_=ot[:])

    nc.compile()
    return nc


def _prep_in_maps(x, W_base, b_base, A, lora_B):
    # Accept jax/np arrays alike; do all host prep in numpy.
    x = np.asarray(x)
    W_base = np.asarray(W_base)
    b_base = np.asarray(b_base)
    A = np.asarray(A)
    lora_B = np.asarray(lora_B)
    # Host prep: exact fold of the LoRA path into the weight.
    W_eff = (W_base.astype(np.float32)
             + SCALING * (lora_B.astype(np.float32) @ A.astype(np.float32)))

    # wT[p, nO, k, o] = W_eff[nO*128+o, k*128+p]
    w_bf = W_eff.astype(ml_dtypes.bfloat16)
    wT = np.ascontiguousarray(
        w_bf.reshape(NO, P, KO, P).transpose(3, 0, 2, 1)
    )

    # bias[p, nO] = b_base[nO*128+p]
    bias_l = np.ascontiguousarray(b_base.astype(np.float32).reshape(NO, P).T)

    xf = x.reshape(BATCH * SEQ, D_IN).astype(ml_dtypes.bfloat16)
    in_maps = []
    for c in range(N_CORES):
        xc = xf[c * S_PER_CORE:(c + 1) * S_PER_CORE]
        # xT[p, mi, k, s] = x_c[mi*512+s, k*128+p]
        xT = np.ascontiguousarray(
            xc.reshape(MI, S_TILE, KO, P).transpose(3, 0, 2, 1)
        )
        in_maps.append({"xT": xT, "wT": wT, "bias": bias_l})
    return in_maps


def _unpack(res):
    out = np.empty((BATCH * SEQ, D_OUT), dtype=np.float32)
    for c in range(N_CORES):
        oc = res.results[c]["out"]  # [P, NO, MI, S_TILE]
        # out_c[mi*512+s, nO*128+p] = oc[p, nO, mi, s]
        out[c * S_PER_CORE:(c + 1) * S_PER_CORE] = (
            oc.transpose(2, 3, 1, 0).reshape(S_PER_CORE, D_OUT)
        )
    return out.reshape(BATCH, SEQ, D_OUT)


def kernel(x, W_base, b_base, A, B):
    lora_B = B
    if "nc" not in _compiled:
        _compiled["nc"] = _build_program()
    nc = _compiled["nc"]
    in_maps = _prep_in_maps(x, W_base, b_base, A, lora_B)
    res = run_bass_kernel_spmd(nc, in_maps, core_ids=list(range(N_CORES)))
    return _unpack(res)


def profiled_run(inputs, tmpdir=None):
    """Re-run the SPMD kernel with NTFF tracing; returns exec_time_ns.
    Used by test.py only (requires the antenv.axon_hooks shim)."""
    if "nc" not in _compiled:
        _compiled["nc"] = _build_program()
    nc = _compiled["nc"]
    in_maps = _prep_in_maps(
        inputs["x"], inputs["W_base"], inputs["b_base"], inputs["A"], inputs["B"]
    )
    res = run_bass_kernel_spmd(
        nc, in_maps, core_ids=list(range(N_CORES)), trace=True, tmpdir=tmpdir
    )
    print("profile tmpdir:", tmpdir)
    return res.exec_time_ns


# revision 17
# speedup vs baseline: 1.0103x; 1.0103x over previous
"""Trainium2 Bass kernel for CascadedLoRALinear4bit.

Computes out[b,s,o] = x @ W_base^T + b_base + scaling * (x @ A^T) @ B^T
with scaling == rank/alpha == 1.0.

Strategy:
  - Algebraic fold (exact): out = x @ (W_base + B @ A)^T + b_base.
    The fold is computed on host in fp32 (0.5 GFLOP, negligible).
  - Data-parallel over tokens: the 4*4096 = 16384 tokens are sharded
    8 ways (2048 tokens per NeuronCore). W_eff^T and bias are
    replicated to all cores. No collectives needed.
  - Per core: out_c^T[4096, 2048] = W_eff @ x_c^T + bias, tiled for
    the PE in bf16 with fp32 PSUM accumulation:
      * x_c^T stays fully resident in SBUF (16 MiB bf16), loaded once.
      * W_eff^T streams through as the stationary operand; each
        stationary tile is reused for 4 moving x chunks.
      * Output is computed transposed (o on partitions) so the bias is
        a per-partition scalar added by the DVE on PSUM eviction.
  - PE roofline: 4096 matmuls x [128x128]@[128x512] bf16.

Layouts (d = contraction dim on partitions everywhere):
  xT  [128, 4, 32, 512]  xT[p,mi,k,s] = x_c[mi*512+s, k*128+p]     (bf16)
  wT  [128, 32, 32, 128] wT[p,nO,k,o] = W_eff[nO*128+o, k*128+p]   (bf16)
  bias[128, 32]          bias[p,nO]   = b_base[nO*128+p]           (f32)
  out [128, 32, 4, 512]  out[p,nO,mi,s] = out_c[mi*512+s, nO*128+p] (f32)
"""

import sys

if "/opt/trn_rl_repo" not in sys.path:
    sys.path.insert(0, "/opt/trn_rl_repo")

import numpy as np
import ml_dtypes

import concourse.bass as bass
import concourse.mybir as mybir
import concourse.tile as tile
from concourse import bacc
from concourse.bass_utils import run_bass_kernel_spmd

# Problem dims (hardcoded per contract)
BATCH, SEQ, D_IN, D_OUT = 4, 4096, 4096, 4096
SCALING = 1.0  # rank / alpha = 16 / 16

N_CORES = 8
P = 128
S_PER_CORE = BATCH * SEQ // N_CORES  # 2048
KO = D_IN // P                       # 32 contraction tiles
S_TILE = 512
MI = S_PER_CORE // S_TILE            # 4 moving (token) chunks
NO = D_OUT // P                      # 32 output-row blocks

BF16 = mybir.dt.bfloat16
F32 = mybir.dt.float32

_compiled = {}


def _build_program(mi_n=MI, no_n=NO, ko=KO, s_tile=S_TILE):
    nc = bacc.Bacc(None, target_bir_lowering=False)

    xT = nc.declare_dram_parameter("xT", [P, mi_n, ko, s_tile], BF16, isOutput=False)
    wT = nc.declare_dram_parameter("wT", [P, no_n, ko, P], BF16, isOutput=False)
    bias_d = nc.declare_dram_parameter("bias", [P, no_n], F32, isOutput=False)
    out_d = nc.declare_dram_parameter("out", [P, no_n, mi_n, s_tile], F32, isOutput=True)

    with tile.TileContext(nc) as tc:
        with (
            tc.tile_pool(name="xres", bufs=1) as x_pool,
            tc.tile_pool(name="wt", bufs=3) as wt_pool,
            tc.tile_pool(name="bias", bufs=1) as bias_pool,
            tc.tile_pool(name="o", bufs=8) as out_pool,
            tc.tile_pool(name="psum", bufs=2, space="PSUM") as psum_pool,
        ):
            bias_t = bias_pool.tile([P, no_n], F32)
            nc.sync.dma_start(out=bias_t[:], in_=bias_d[:])

            # First stationary block, then x_c^T preload in k-major chunk
            # order so chunks land in the order the nO=0 k-loop consumes
            # them (x stays fully resident for all later nO iterations).
            K_CHUNK = min(2, ko)
            wt0 = wt_pool.tile([P, ko, P], BF16, name="wt")
            nc.sync.dma_start(out=wt0[:], in_=wT[:, 0, :, :])

            xres = [x_pool.tile([P, ko, s_tile], BF16, name=f"x{mi}")
                    for mi in range(mi_n)]
            for kc in range(0, ko, K_CHUNK):
                for mi in range(mi_n):
                    nc.sync.dma_start(
                        out=xres[mi][:, kc:kc + K_CHUNK, :],
                        in_=xT[:, mi, kc:kc + K_CHUNK, :],
                    )

            for n in range(no_n):
                if n == 0:
                    wt_blk = wt0
                else:
                    wt_blk = wt_pool.tile([P, ko, P], BF16, name="wt")
                    nc.sync.dma_start(out=wt_blk[:], in_=wT[:, n, :, :])
                pss = [psum_pool.tile([P, s_tile], F32, name=f"ps{mi}")
                       for mi in range(mi_n)]
                for k in range(ko):
                    for mi in range(mi_n):
                        nc.tensor.matmul(
                            pss[mi][:],
                            lhsT=wt_blk[:, k, :],
                            rhs=xres[mi][:, k, :],
                            start=(k == 0),
                            stop=(k == ko - 1),
                        )
                for mi in range(mi_n):
                    ot = out_pool.tile([P, s_tile], F32)
                    nc.vector.tensor_scalar_add(ot[:], pss[mi][:], bias_t[:, n:n + 1])
                    nc.sync.dma_start(out=out_d[:, n, mi, :], in_=ot[:])

    nc.compile()
    return nc


def _prep_in_maps(x, W_base, b_base, A, lora_B):
    # Accept jax/np arrays alike; do all host prep in numpy.
    x = np.asarray(x)
    W_base = np.asarray(W_base)
    b_base = np.asarray(b_base)
    A = np.asarray(A)
    lora_B = np.asarray(lora_B)
    # Host prep: exact fold of the LoRA path into the weight.
    W_eff = (W_base.astype(np.float32)
             + SCALING * (lora_B.astype(np.float32) @ A.astype(np.float32)))

    # wT[p, nO, k, o] = W_eff[nO*128+o, k*128+p]
    w_bf = W_eff.astype(ml_dtypes.bfloat16)
    wT = np.ascontiguousarray(
        w_bf.reshape(NO, P, KO, P).transpose(3, 0, 2, 1)
    )

    # bias[p, nO] = b_base[nO*128+p]
    bias_l = np.ascontiguousarray(b_base.astype(np.float32).reshape(NO, P).T)

    xf = x.reshape(BATCH * SEQ, D_IN).astype(ml_dtypes.bfloat16)
    in_maps = []
    for c in range(N_CORES):
        xc = xf[c * S_PER_CORE:(c + 1) * S_PER_CORE]
        # xT[p, mi, k, s] = x_c[mi*512+s, k*128+p]
        xT = np.ascontiguousarray(
            xc.reshape(MI, S_TILE, KO, P).transpose(3, 0, 2, 1)
        )
        in_maps.append({"xT": xT, "wT": wT, "bias": bias_l})
    return in_maps


def _unpack(res):
    out = np.empty((BATCH * SEQ, D_OUT), dtype=np.float32)
    for c in range(N_CORES):
        oc = res.results[c]["out"]  # [P, NO, MI, S_TILE]
        # out_c[mi*512+s, nO*128+p] = oc[p, nO, mi, s]
        out[c * S_PER_CORE:(c + 1) * S_PER_CORE] = (
            oc.transpose(2, 3, 1, 0).reshape(S_PER_CORE, D_OUT)
        )
    return out.reshape(BATCH, SEQ, D_OUT)


def kernel(x, W_base, b_base, A, B):
    lora_B = B
    if "nc" not in _compiled:
        _compiled["nc"] = _build_program()
    nc = _compiled["nc"]
    in_maps = _prep_in_maps(x, W_base, b_base, A, lora_B)
    res = run_bass_kernel_spmd(nc, in_maps, core_ids=list(range(N_CORES)))
    return _unpack(res)


def profiled_run(inputs, tmpdir=None, trace_cores=None):
    """Re-run the SPMD kernel with NTFF tracing; returns exec_time_ns
    (max across traced cores). Used by test.py only (requires the
    antenv.axon_hooks shim)."""
    if "nc" not in _compiled:
        _compiled["nc"] = _build_program()
    nc = _compiled["nc"]
    in_maps = _prep_in_maps(
        inputs["x"], inputs["W_base"], inputs["b_base"], inputs["A"], inputs["B"]
    )
    res = run_bass_kernel_spmd(
        nc, in_maps, core_ids=list(range(N_CORES)), trace=True, tmpdir=tmpdir,
        trace_cores=trace_cores,
    )
    print("profile tmpdir:", tmpdir)
    if res.mean_exec_time_ns is not None:
        print(f"mean exec across traced cores: {res.mean_exec_time_ns:.0f} ns; "
              f"slowest core: {res.max_exec_time_core_id}")
    return res.exec_time_ns
